# revision 9
# baseline (speedup 1.0000x reference)
"""Trainium2 Bass kernel for nn_EqModelComplex (complex-valued pre-LN transformer
block: complex LN -> complex QKV -> RoPE -> causal attn (Re Hermitian scores)
-> complex out-proj -> residual -> complex LN -> complex FFN w/ ModReLU -> residual).

Sharding over 8 NeuronCores:
  - Attention is head-sharded (16 heads -> 2 per core).
  - LN1/LN2, out-proj, FFN and residuals are token-sharded (2048 tokens -> 256/core).
  - Connected by one AllGather (LN1 output, so every core sees all tokens for
    its heads' QKV) and one AllToAll (attention head outputs -> token shards).
  - LN gamma/beta are folded into the adjacent projection weights on the host;
    r/i complex parts are stacked into the partition dim so scores/out-proj
    contractions fuse the real+imag products into single matmuls.

All activations live transposed on-device: [feature, token]. All matmul
operands are fp16 (fp32 PSUM accumulation); the residual stream is fp32.

Self-contained: hardcodes shapes; builds + compiles the Bass graph on first
call and runs via run_bass_kernel_spmd on cores 0-7.
"""

import os
import sys

sys.path.insert(0, "/opt/trn_rl_repo")

import numpy as np

import concourse.bass as bass
import concourse.bacc as bacc
import concourse.tile as tile
from concourse import mybir
from concourse.bass_utils import run_bass_kernel_spmd

# ---------------- problem dims ----------------
B, L, D, H = 2, 1024, 1024, 16
HD = D // H                  # 64
HIDDEN = 4 * D               # 4096
EPS = 1e-6
SCALE = HD ** -0.5
NC = 8                       # cores
T_ALL = B * L                # 2048 tokens
TOK = T_ALL // NC            # 256 tokens per core
KT = D // 128                # 8 k-tiles over D
HB = HIDDEN // 128           # 32 h-blocks over HIDDEN
OB = D // 128                # 8 out-blocks over D
HPC = H // NC                # 2 heads per core

F16 = mybir.dt.float16
F32 = mybir.dt.float32
AF = mybir.ActivationFunctionType
OP = mybir.AluOpType

_cache = {}


# =====================================================================
# Device kernel emission
# =====================================================================
def _emit(tc, T):
    nc = tc.nc
    ctx_pools = []

    def dram_pool():
        return tc.tile_pool(name="dram", bufs=1, space="DRAM")

    import contextlib

    with contextlib.ExitStack() as ES:
        const = ES.enter_context(tc.tile_pool(name="const", bufs=1))
        dram = ES.enter_context(tc.tile_pool(name="dramp", bufs=1, space="DRAM"))

        # ---------------- constants to SBUF ----------------
        cos_sb = const.tile([128, L], F16, name="cos_sb")
        sin_sb = const.tile([128, L], F16, name="sin_sb")
        nc.sync.dma_start(cos_sb[:], T["cos2"][:])
        nc.sync.dma_start(sin_sb[:], T["sin2"][:])
        mask_sb = const.tile([128, 128], F16, name="mask_sb")
        nc.sync.dma_start(mask_sb[:], T["mask01"][:])
        ones16 = const.tile([128, 1], F16, name="ones16")
        nc.vector.memset(ones16[:], 1.0)
        ones32 = const.tile([1, 128], F32, name="ones32")
        nc.vector.memset(ones32[:], 1.0)
        qb_sb = const.tile([128, 2], F32, name="qb_sb")
        kb_sb = const.tile([128, 2], F32, name="kb_sb")
        nc.sync.dma_start(qb_sb[:], T["qbias"][:])
        nc.sync.dma_start(kb_sb[:], T["kbias"][:])
        vb_sb = const.tile([128, 2 * 128], F32, name="vb_sb")
        nc.sync.dma_start(vb_sb[:], T["vbias_bc"][:])
        ob_r_sb = const.tile([128, OB], F32, name="ob_r_sb")
        ob_i_sb = const.tile([128, OB], F32, name="ob_i_sb")
        nc.sync.dma_start(ob_r_sb[:], T["obias_r"][:])
        nc.sync.dma_start(ob_i_sb[:], T["obias_i"][:])
        b1r_sb = const.tile([128, HB], F32, name="b1r_sb")
        b1i_sb = const.tile([128, HB], F32, name="b1i_sb")
        modb_sb = const.tile([128, HB], F32, name="modb_sb")
        nc.sync.dma_start(b1r_sb[:], T["bias1_r"][:])
        nc.sync.dma_start(b1i_sb[:], T["bias1_i"][:])
        nc.sync.dma_start(modb_sb[:], T["modb"][:])
        b2r_sb = const.tile([128, OB], F32, name="b2r_sb")
        b2i_sb = const.tile([128, OB], F32, name="b2i_sb")
        nc.sync.dma_start(b2r_sb[:], T["bias2_r"][:])
        nc.sync.dma_start(b2i_sb[:], T["bias2_i"][:])

        # internal DRAM comm buffers
        ag1_in = dram.tile([2, D, TOK], F16, name="ag1_in")
        ag1_out = dram.tile([NC, 2, D, TOK], F16, name="ag1_out", addr_space="Shared")
        a2a_in = dram.tile([NC, 2 * 128, TOK], F16, name="a2a_in")
        a2a_out = dram.tile([NC, 2 * 128, TOK], F16, name="a2a_out")

        # =====================================================
        # complex layer norm (shared by LN1 / LN2)
        #   xr/xi: [128, KT, TOK] f32 SBUF -> writes hn tiles (fp16)
        # =====================================================
        def complex_ln(xr, xi, outs, lnp, lnps, tagp):
            # casts to fp16 + squares
            xr16 = lnp.tile([128, KT, TOK], F16, name=f"xr16{tagp}")
            xi16 = lnp.tile([128, KT, TOK], F16, name=f"xi16{tagp}")
            nc.vector.tensor_copy(xr16[:], xr[:])
            nc.vector.tensor_copy(xi16[:], xi[:])
            sq = lnp.tile([128, KT, TOK], F16, name=f"sq{tagp}")
            t2 = lnp.tile([128, KT, TOK], F16, name=f"t2{tagp}")
            nc.vector.tensor_tensor(sq[:], xr16[:], xr16[:], OP.mult)
            nc.vector.tensor_tensor(t2[:], xi16[:], xi16[:], OP.mult)
            nc.vector.tensor_tensor(sq[:], sq[:], t2[:], OP.add)
            # stats matmuls: sum over D (partition dim) via ones
            ps_mr = lnps.tile([1, TOK], F32, name=f"psmr{tagp}", tag=f"psmr{tagp}")
            ps_mi = lnps.tile([1, TOK], F32, name=f"psmi{tagp}", tag=f"psmi{tagp}")
            ps_sq = lnps.tile([1, TOK], F32, name=f"pssq{tagp}", tag=f"pssq{tagp}")
            for kt in range(KT):
                nc.tensor.matmul(ps_mr[:], ones16[:], xr16[:, kt, :],
                                 start=(kt == 0), stop=(kt == KT - 1))
                nc.tensor.matmul(ps_mi[:], ones16[:], xi16[:, kt, :],
                                 start=(kt == 0), stop=(kt == KT - 1))
                nc.tensor.matmul(ps_sq[:], ones16[:], sq[:, kt, :],
                                 start=(kt == 0), stop=(kt == KT - 1))
            mr = lnp.tile([1, TOK], F32, name=f"mr{tagp}")
            mi = lnp.tile([1, TOK], F32, name=f"mi{tagp}")
            msq = lnp.tile([1, TOK], F32, name=f"msq{tagp}")
            inv_d = 1.0 / D
            nc.scalar.mul(mr[:], ps_mr[:], inv_d)
            nc.scalar.mul(mi[:], ps_mi[:], inv_d)
            nc.scalar.mul(msq[:], ps_sq[:], inv_d)
            # var = msq - mr^2 - mi^2 ; rstd = 1/sqrt(var+eps)
            v1 = lnp.tile([1, TOK], F32, name=f"v1{tagp}")
            nc.vector.tensor_tensor(v1[:], mr[:], mr[:], OP.mult)
            nc.vector.tensor_tensor(v1[:], msq[:], v1[:], OP.subtract)
            v2 = lnp.tile([1, TOK], F32, name=f"v2{tagp}")
            nc.vector.tensor_tensor(v2[:], mi[:], mi[:], OP.mult)
            nc.vector.tensor_tensor(v1[:], v1[:], v2[:], OP.subtract)
            nc.vector.tensor_scalar_add(v1[:], v1[:], EPS)
            # rstd = 1/sqrt(var+eps) = exp(-0.5*ln(var+eps)); keeps every ACT
            # func in the natural_log_exp_and_others table set (no reloads)
            rv = lnp.tile([1, TOK], F32, name=f"rv{tagp}")
            nc.scalar.activation(rv[:], v1[:], AF.Ln)
            rstd = lnp.tile([1, TOK], F32, name=f"rstd{tagp}")
            nc.scalar.activation(rstd[:], rv[:], AF.Exp, scale=-0.5)
            # broadcast mr, mi, rstd to 128 partitions via K=1 fp32 matmul
            ps_bc = lnps.tile([128, 2 * TOK], F32, name=f"psbc{tagp}", tag=f"psbc{tagp}")
            nc.tensor.matmul(ps_bc[:, 0:TOK], ones32[:], mr[:], start=True, stop=True)
            nc.tensor.matmul(ps_bc[:, TOK:2 * TOK], ones32[:], mi[:], start=True, stop=True)
            ps_bc2 = lnps.tile([128, TOK], F32, name=f"psbc2{tagp}", tag=f"psbc2{tagp}")
            nc.tensor.matmul(ps_bc2[:], ones32[:], rstd[:], start=True, stop=True)
            bc_m = lnp.tile([128, 2 * TOK], F32, name=f"bcm{tagp}")
            bc_s = lnp.tile([128, TOK], F32, name=f"bcs{tagp}")
            nc.scalar.copy(bc_m[:], ps_bc[:])
            nc.scalar.copy(bc_s[:], ps_bc2[:])
            # normalize: hn = (x - m) * rstd  (fp16 out)
            for kt in range(KT):
                tr = lnp.tile([128, TOK], F32, name=f"tr{tagp}", tag=f"tr{tagp}", bufs=2)
                nc.vector.tensor_tensor(tr[:], xr[:, kt, :], bc_m[:, 0:TOK], OP.subtract)
                nc.vector.tensor_tensor(outs["hnr"][:, kt, :], tr[:], bc_s[:], OP.mult)
                ti = lnp.tile([128, TOK], F32, name=f"ti{tagp}", tag=f"ti{tagp}", bufs=2)
                nc.vector.tensor_tensor(ti[:], xi[:, kt, :], bc_m[:, TOK:2 * TOK], OP.subtract)
                nc.vector.tensor_tensor(outs["hni"][:, kt, :], ti[:], bc_s[:], OP.mult)
                if "hnin" in outs:
                    nc.vector.tensor_scalar_mul(outs["hnin"][:, kt, :],
                                                outs["hni"][:, kt, :], -1.0)

        # =====================================================
        # Phase 1: LN1 on this core's 256 tokens, then AllGather
        # =====================================================
        with tc.tile_pool(name="ln1", bufs=1) as lnp, \
             tc.tile_pool(name="ln1ps", bufs=1, space="PSUM") as lnps:
            xr_sb = lnp.tile([128, KT, TOK], F32, name="xr_sb")
            xi_sb = lnp.tile([128, KT, TOK], F32, name="xi_sb")
            for kt in range(KT):
                nc.sync.dma_start(xr_sb[:, kt, :], T["xT_r"][128 * kt:128 * (kt + 1), :])
                nc.sync.dma_start(xi_sb[:, kt, :], T["xT_i"][128 * kt:128 * (kt + 1), :])
            hnr_loc = lnp.tile([128, KT, TOK], F16, name="hnr_loc")
            hni_loc = lnp.tile([128, KT, TOK], F16, name="hni_loc")
            complex_ln(xr_sb, xi_sb, {"hnr": hnr_loc, "hni": hni_loc}, lnp, lnps, "1")
            for kt in range(KT):
                nc.sync.dma_start(ag1_in[0, 128 * kt:128 * (kt + 1), :], hnr_loc[:, kt, :])
                nc.sync.dma_start(ag1_in[1, 128 * kt:128 * (kt + 1), :], hni_loc[:, kt, :])
            nc.gpsimd.collective_compute(
                "AllGather", OP.bypass,
                replica_groups=[list(range(NC))],
                ins=[ag1_in.opt()], outs=[ag1_out.opt()],
            )

        # =====================================================
        # Phase 2: QKV projections (heads 2c, 2c+1) + RoPE
        # =====================================================
        attn_scope = contextlib.ExitStack()
        attn = attn_scope.enter_context(tc.tile_pool(name="attn", bufs=1))
        # gathered hn, all 2048 tokens, as matmul moving operands
        hnr_mm = [attn.tile([128, T_ALL], F16, name=f"hnr_mm{kt}") for kt in range(KT)]
        hni_mm = [attn.tile([128, T_ALL], F16, name=f"hni_mm{kt}") for kt in range(KT)]
        for kt in range(KT):
            for r in range(NC):
                nc.sync.dma_start(hnr_mm[kt][:, TOK * r:TOK * (r + 1)],
                                  ag1_out[r, 0, 128 * kt:128 * (kt + 1), :])
                nc.sync.dma_start(hni_mm[kt][:, TOK * r:TOK * (r + 1)],
                                  ag1_out[r, 1, 128 * kt:128 * (kt + 1), :])

        # weights
        wq_a = [attn.tile([128, KT, 128], F16, name=f"wq_a{h}") for h in range(HPC)]
        wq_b = [attn.tile([128, KT, 128], F16, name=f"wq_b{h}") for h in range(HPC)]
        wk_a = [attn.tile([128, KT, 128], F16, name=f"wk_a{h}") for h in range(HPC)]
        wk_b = [attn.tile([128, KT, 128], F16, name=f"wk_b{h}") for h in range(HPC)]
        for hh in range(HPC):
            nc.sync.dma_start(wq_a[hh][:], T["wq_a"][hh].rearrange("kt k j -> k kt j"))
            nc.sync.dma_start(wq_b[hh][:], T["wq_b"][hh].rearrange("kt k j -> k kt j"))
            nc.sync.dma_start(wk_a[hh][:], T["wk_a"][hh].rearrange("kt k j -> k kt j"))
            nc.sync.dma_start(wk_b[hh][:], T["wk_b"][hh].rearrange("kt k j -> k kt j"))
        wv_a = attn.tile([128, KT, 2 * 128], F16, name="wv_a")
        wv_b = attn.tile([128, KT, 2 * 128], F16, name="wv_b")
        nc.sync.dma_start(wv_a[:], T["wv_a"].rearrange("kt k j -> k kt j"))
        nc.sync.dma_start(wv_b[:], T["wv_b"].rearrange("kt k j -> k kt j"))

        # persistent fp16 Q/K (post-RoPE, r/i stacked per head) and V
        qbf = [attn.tile([128, T_ALL], F16, name=f"qbf{h}") for h in range(HPC)]
        kbf = [attn.tile([128, T_ALL], F16, name=f"kbf{h}") for h in range(HPC)]
        v_sb = attn.tile([128, 2 * NC, 2 * 128], F16, name="v_sb")  # [t128, ttile, (h,ri)]

        def rope(dst, src, rp):
            # dst = src*cos + shift(src)*sin   (fp16 [128, 2048])
            sh = rp.tile([128, T_ALL], F16, name="sh", tag="rope_sh", bufs=2)
            for base in (0, 64):
                nc.sync.dma_start(sh[base:base + 32, :], src[base + 32:base + 64, :])
                nc.sync.dma_start(sh[base + 32:base + 64, :], src[base:base + 32, :])
            t1 = rp.tile([128, T_ALL], F16, name="t1", tag="rope_t1", bufs=2)
            c3 = cos_sb[:, None, :].to_broadcast((128, B, L))
            s3 = sin_sb[:, None, :].to_broadcast((128, B, L))
            src3 = src.rearrange("p (b l) -> p b l", b=B)
            sh3 = sh.rearrange("p (b l) -> p b l", b=B)
            t13 = t1.rearrange("p (b l) -> p b l", b=B)
            dst3 = dst.rearrange("p (b l) -> p b l", b=B)
            nc.vector.tensor_tensor(t13, src3, c3, OP.mult)
            nc.vector.tensor_tensor(sh3, sh3, s3, OP.mult)
            nc.vector.tensor_tensor(dst3, t13, sh3, OP.add)

        with tc.tile_pool(name="qkps", bufs=1, space="PSUM") as qkps, \
             tc.tile_pool(name="ropep", bufs=1) as rp:
            for hh in range(HPC):
                for which, wa, wb, bias_col, dst in (
                        ("q", wq_a[hh], wq_b[hh], qb_sb[:, hh:hh + 1], qbf[hh]),
                        ("k", wk_a[hh], wk_b[hh], kb_sb[:, hh:hh + 1], kbf[hh])):
                    tmp = rp.tile([128, T_ALL], F16, name=f"tmp{which}{hh}",
                                  tag="qktmp", bufs=2)
                    for half in range(2):
                        ps = qkps.tile([128, 1024], F32, name=f"qk{which}{hh}{half}",
                                       tag="qkps", bufs=2)
                        for ch in range(2):
                            lo = 1024 * half + 512 * ch
                            for kt in range(KT):
                                nc.tensor.matmul(ps[:, 512 * ch:512 * (ch + 1)],
                                                 wa[:, kt, :], hnr_mm[kt][:, lo:lo + 512],
                                                 start=(kt == 0), stop=False)
                            for kt in range(KT):
                                nc.tensor.matmul(ps[:, 512 * ch:512 * (ch + 1)],
                                                 wb[:, kt, :], hni_mm[kt][:, lo:lo + 512],
                                                 start=False, stop=(kt == KT - 1))
                        nc.scalar.activation(tmp[:, 1024 * half:1024 * (half + 1)],
                                             ps[:], AF.Identity, bias=bias_col)
                    rope(dst, tmp, rp)

            # V projection: stationary hn tiles, moving wv covering both heads
            for tt in range(2 * NC):
                vps = qkps.tile([128, 2 * 128], F32, name=f"vps{tt}", tag="vps", bufs=2)
                for kt in range(KT):
                    nc.tensor.matmul(vps[:], hnr_mm[kt][:, 128 * tt:128 * (tt + 1)],
                                     wv_a[:, kt, :], start=(kt == 0), stop=False)
                for kt in range(KT):
                    nc.tensor.matmul(vps[:], hni_mm[kt][:, 128 * tt:128 * (tt + 1)],
                                     wv_b[:, kt, :], start=False, stop=(kt == KT - 1))
                nc.vector.tensor_tensor(v_sb[:, tt, :], vps[:], vb_sb[:], OP.add)

        # =====================================================
        # Phase 3: attention per (batch, head): S^T -> exp -> mask
        #          -> row sums + A@V -> normalize
        # =====================================================
        ot_sb = [attn.tile([128, T_ALL], F16, name=f"ot_sb{h}") for h in range(HPC)]
        NB = L // 128  # 8 m-blocks per batch

        with tc.tile_pool(name="stps", bufs=1, space="PSUM") as stps, \
             tc.tile_pool(name="otps", bufs=1, space="PSUM") as otps, \
             tc.tile_pool(name="smps", bufs=1, space="PSUM") as smps, \
             tc.tile_pool(name="atw", bufs=1) as atw:
            for b in range(B):
                t0 = L * b
                for hh in range(HPC):
                    pts = []
                    for kb in range(NB):
                        lo = 128 * kb
                        st = stps.tile([128, L], F32, name=f"st{b}{hh}{kb}",
                                       tag="st", bufs=2)
                        pieces = []
                        if lo < 512:
                            pieces.append((lo, 512))
                            pieces.append((512, 1024))
                        else:
                            pieces.append((lo, 1024))
                        for (a, e) in pieces:
                            nc.tensor.matmul(st[:, a:e],
                                             kbf[hh][:, t0 + lo:t0 + lo + 128],
                                             qbf[hh][:, t0 + a:t0 + e],
                                             start=True, stop=True)
                        pt = atw.tile([128, L], F16, name=f"pt{b}{hh}{kb}",
                                      tag="pt", bufs=8)
                        nc.scalar.activation(pt[:, lo:L], st[:, lo:L], AF.Exp)
                        nc.vector.tensor_tensor(pt[:, lo:lo + 128], pt[:, lo:lo + 128],
                                                mask_sb[:], OP.mult)
                        pts.append((kb, lo, pt))

                    ot = otps.tile([128, L], F32, name=f"ot{b}{hh}", tag="ot", bufs=1)
                    sm = smps.tile([1, L], F32, name=f"sm{b}{hh}", tag="sm", bufs=1)
                    for kb, lo, pt in pts:
                        vstat = v_sb[:, NB * b + kb, 128 * hh:128 * (hh + 1)]
                        pieces = []
                        if lo < 512:
                            pieces.append((lo, 512, kb == 0, kb == 3))
                            pieces.append((512, 1024, kb == 0, kb == NB - 1))
                        else:
                            pieces.append((lo, 1024, False, kb == NB - 1))
                        for (a, e, st_, sp_) in pieces:
                            nc.tensor.matmul(ot[:, a:e], vstat, pt[:, a:e],
                                             start=st_, stop=sp_)
                            nc.tensor.matmul(sm[:, a:e], ones16[:], pt[:, a:e],
                                             start=st_, stop=sp_)
                    # normalize columns by 1/rowsum
                    sm_sb = atw.tile([1, L], F32, name=f"smsb{b}{hh}", tag="smsb", bufs=2)
                    nc.scalar.copy(sm_sb[:], sm[:])
                    rc = atw.tile([1, L], F32, name=f"rc{b}{hh}", tag="rc", bufs=2)
                    nc.vector.reciprocal(rc[:], sm_sb[:])
                    raw = atw.tile([128, L], F16, name=f"raw{b}{hh}", tag="raw", bufs=2)
                    nc.scalar.copy(raw[:], ot[:])
                    bc = otps.tile([128, L], F32, name=f"bc{b}{hh}", tag="ot", bufs=1)
                    nc.tensor.matmul(bc[:, 0:512], ones32[:], rc[:, 0:512],
                                     start=True, stop=True)
                    nc.tensor.matmul(bc[:, 512:1024], ones32[:], rc[:, 512:1024],
                                     start=True, stop=True)
                    bc_sb = atw.tile([128, L], F32, name=f"bcsb{b}{hh}", tag="bcsb", bufs=2)
                    nc.scalar.copy(bc_sb[:], bc[:])
                    nc.vector.tensor_tensor(ot_sb[hh][:, t0:t0 + L], raw[:], bc_sb[:],
                                            OP.mult)

        # AllToAll: [slot j] = OT[:, 256j:256j+256] -> core j gets all heads for its tokens
        for j in range(NC):
            for hh in range(HPC):
                nc.sync.dma_start(a2a_in[j, 128 * hh:128 * (hh + 1), :],
                                  ot_sb[hh][:, TOK * j:TOK * (j + 1)])
        nc.gpsimd.collective_compute(
            "AllToAll", OP.bypass,
            replica_groups=[list(range(NC))],
            ins=[a2a_in.opt()], outs=[a2a_out.opt()],
        )
        attn_scope.close()  # free attention SBUF before the FFN half

        # =====================================================
        # Phase 4: out-projection (token-parallel) + residual -> ar
        # =====================================================
        ffn = ES.enter_context(tc.tile_pool(name="ffn", bufs=1))
        ar_sb = ffn.tile([128, OB, TOK], F32, name="ar_sb")
        ai_sb = ffn.tile([128, OB, TOK], F32, name="ai_sb")

        with tc.tile_pool(name="opw", bufs=1) as opw, \
             tc.tile_pool(name="opps", bufs=2, space="PSUM") as opps:
            og = [opw.tile([128, TOK], F16, name=f"og{h}") for h in range(H)]
            for h in range(H):
                nc.sync.dma_start(og[h][:],
                                  a2a_out[h // 2, 128 * (h % 2):128 * (h % 2 + 1), :])
            wo_c = opw.tile([128, H, D], F16, name="wo_c")
            wo_d = opw.tile([128, H, D], F16, name="wo_d")
            nc.sync.dma_start(wo_c[:], T["wo_c"].rearrange("h k j -> k h j"))
            nc.sync.dma_start(wo_d[:], T["wo_d"].rearrange("h k j -> k h j"))
            # x^T reload for the residual
            x2r = opw.tile([128, OB, TOK], F32, name="x2r")
            x2i = opw.tile([128, OB, TOK], F32, name="x2i")
            for kt in range(OB):
                nc.sync.dma_start(x2r[:, kt, :], T["xT_r"][128 * kt:128 * (kt + 1), :])
                nc.sync.dma_start(x2i[:, kt, :], T["xT_i"][128 * kt:128 * (kt + 1), :])
            for obk in range(OB):
                osl = slice(128 * obk, 128 * (obk + 1))
                pr = opps.tile([128, TOK], F32, name=f"pr{obk}", tag="opr", bufs=2)
                pi = opps.tile([128, TOK], F32, name=f"pi{obk}", tag="opi", bufs=2)
                for h in range(H):
                    nc.tensor.matmul(pr[:], wo_c[:, h, osl], og[h][:],
                                     start=(h == 0), stop=(h == H - 1))
                for h in range(H):
                    nc.tensor.matmul(pi[:], wo_d[:, h, osl], og[h][:],
                                     start=(h == 0), stop=(h == H - 1))
                nc.vector.scalar_tensor_tensor(ar_sb[:, obk, :], pr[:],
                                               ob_r_sb[:, obk:obk + 1], x2r[:, obk, :],
                                               OP.add, OP.add)
                nc.vector.scalar_tensor_tensor(ai_sb[:, obk, :], pi[:],
                                               ob_i_sb[:, obk:obk + 1], x2i[:, obk, :],
                                               OP.add, OP.add)

        # =====================================================
        # Phase 5: LN2 (token-parallel)
        # =====================================================
        hn2r = ffn.tile([128, KT, TOK], F16, name="hn2r")
        hn2i = ffn.tile([128, KT, TOK], F16, name="hn2i")
        hn2in = ffn.tile([128, KT, TOK], F16, name="hn2in")
        with tc.tile_pool(name="ln2", bufs=1) as lnp2, \
             tc.tile_pool(name="ln2ps", bufs=1, space="PSUM") as lnps2:
            complex_ln(ar_sb, ai_sb, {"hnr": hn2r, "hni": hn2i, "hnin": hn2in},
                       lnp2, lnps2, "2")

        # =====================================================
        # Phase 6: fc1 + ModReLU  -> f' tiles (fp16)
        # =====================================================
        fpr = [ffn.tile([128, TOK], F16, name=f"fpr{hb}") for hb in range(HB)]
        fpi = [ffn.tile([128, TOK], F16, name=f"fpi{hb}") for hb in range(HB)]
        with tc.tile_pool(name="f1w", bufs=8) as f1w, \
             tc.tile_pool(name="mrw", bufs=3) as mrw, \
             tc.tile_pool(name="f1ps", bufs=2, space="PSUM") as f1ps:
            for hb in range(HB):
                hsl = slice(128 * hb, 128 * (hb + 1))
                w1r = [f1w.tile([128, 128], F16, name=f"w1r{hb}_{kt}", tag="w1r")
                       for kt in range(KT)]
                w1i = [f1w.tile([128, 128], F16, name=f"w1i{hb}_{kt}", tag="w1i")
                       for kt in range(KT)]
                for kt in range(KT):
                    nc.sync.dma_start(w1r[kt][:], T["w1_r"][kt, :, hsl])
                    nc.sync.dma_start(w1i[kt][:], T["w1_i"][kt, :, hsl])
                fr = f1ps.tile([128, TOK], F32, name=f"fr{hb}", tag="fr", bufs=2)
                fi = f1ps.tile([128, TOK], F32, name=f"fi{hb}", tag="fi", bufs=2)
                for kt in range(KT):
                    nc.tensor.matmul(fr[:], w1r[kt][:], hn2r[:, kt, :],
                                     start=(kt == 0), stop=False)
                    nc.tensor.matmul(fi[:], w1i[kt][:], hn2r[:, kt, :],
                                     start=(kt == 0), stop=False)
                for kt in range(KT):
                    nc.tensor.matmul(fr[:], w1i[kt][:], hn2in[:, kt, :],
                                     start=False, stop=(kt == KT - 1))
                    nc.tensor.matmul(fi[:], w1r[kt][:], hn2i[:, kt, :],
                                     start=False, stop=(kt == KT - 1))
                # ModReLU: m=|f+b|; g=relu(1 + modb/m); f' = (f+b)*g
                bcr = b1r_sb[:, hb:hb + 1]
                bci = b1i_sb[:, hb:hb + 1]
                sq1 = mrw.tile([128, TOK], F32, name=f"sq1_{hb}", tag="sq1")
                sq2 = mrw.tile([128, TOK], F32, name=f"sq2_{hb}", tag="sq2")
                nc.scalar.activation(sq1[:], fr[:], AF.Square, bias=bcr)
                nc.scalar.activation(sq2[:], fi[:], AF.Square, bias=bci)
                nc.vector.tensor_tensor(sq1[:], sq1[:], sq2[:], OP.add)
                # 1/|z| = exp(-0.5*ln(|z|^2))
                rs = mrw.tile([128, TOK], F32, name=f"rs_{hb}", tag="rs")
                nc.scalar.activation(rs[:], sq1[:], AF.Ln)
                rm = mrw.tile([128, TOK], F32, name=f"rm_{hb}", tag="rm")
                nc.scalar.activation(rm[:], rs[:], AF.Exp, scale=-0.5)
                g = mrw.tile([128, TOK], F32, name=f"g_{hb}", tag="g")
                nc.scalar.activation(g[:], rm[:], AF.Relu, bias=1.0,
                                     scale=modb_sb[:, hb:hb + 1])
                nc.vector.scalar_tensor_tensor(fpr[hb][:], fr[:], bcr, g[:],
                                               OP.add, OP.mult)
                nc.vector.scalar_tensor_tensor(fpi[hb][:], fi[:], bci, g[:],
                                               OP.add, OP.mult)

        # =====================================================
        # Phase 7: fc2 + residual -> output
        # =====================================================
        with tc.tile_pool(name="f2w", bufs=24) as f2w, \
             tc.tile_pool(name="f2ps", bufs=2, space="PSUM") as f2ps, \
             tc.tile_pool(name="outp", bufs=4) as outp:
            for obk in range(OB):
                osl = slice(128 * obk, 128 * (obk + 1))
                orr = f2ps.tile([128, TOK], F32, name=f"orr{obk}", tag="orr", bufs=2)
                oii = f2ps.tile([128, TOK], F32, name=f"oii{obk}", tag="oii", bufs=2)
                for hk in range(HB):
                    w2r = f2w.tile([128, 128], F16, name=f"w2r{obk}_{hk}", tag="w2r")
                    w2i = f2w.tile([128, 128], F16, name=f"w2i{obk}_{hk}", tag="w2i")
                    w2n = f2w.tile([128, 128], F16, name=f"w2n{obk}_{hk}", tag="w2n")
                    nc.sync.dma_start(w2r[:], T["w2_r"][hk, :, osl])
                    nc.sync.dma_start(w2i[:], T["w2_i"][hk, :, osl])
                    nc.sync.dma_start(w2n[:], T["w2_i_neg"][hk, :, osl])
                    nc.tensor.matmul(orr[:], w2r[:], fpr[hk][:],
                                     start=(hk == 0), stop=False)
                    nc.tensor.matmul(oii[:], w2i[:], fpr[hk][:],
                                     start=(hk == 0), stop=False)
                    nc.tensor.matmul(orr[:], w2n[:], fpi[hk][:],
                                     start=False, stop=(hk == HB - 1))
                    nc.tensor.matmul(oii[:], w2r[:], fpi[hk][:],
                                     start=False, stop=(hk == HB - 1))
                out_r = outp.tile([128, TOK], F32, name=f"outr{obk}", tag="outr", bufs=2)
                out_i = outp.tile([128, TOK], F32, name=f"outi{obk}", tag="outi", bufs=2)
                nc.vector.scalar_tensor_tensor(out_r[:], orr[:], b2r_sb[:, obk:obk + 1],
                                               ar_sb[:, obk, :], OP.add, OP.add)
                nc.vector.scalar_tensor_tensor(out_i[:], oii[:], b2i_sb[:, obk:obk + 1],
                                               ai_sb[:, obk, :], OP.add, OP.add)
                nc.sync.dma_start(T["outT_r"][osl, :], out_r[:])
                nc.sync.dma_start(T["outT_i"][osl, :], out_i[:])


# =====================================================================
# Graph build + compile (cached)
# =====================================================================
def _build():
    nc = bacc.Bacc("TRN2", target_bir_lowering=False, debug=False,
                   enable_asserts=False, num_devices=NC)
    T = {}

    def inp(name, shape, dt=F16):
        T[name] = nc.dram_tensor(name, list(shape), dt, kind="ExternalInput")

    inp("xT_r", (D, TOK), F32)
    inp("xT_i", (D, TOK), F32)
    inp("wq_a", (HPC, KT, 128, 128))
    inp("wq_b", (HPC, KT, 128, 128))
    inp("wk_a", (HPC, KT, 128, 128))
    inp("wk_b", (HPC, KT, 128, 128))
    inp("wv_a", (KT, 128, 2 * 128))
    inp("wv_b", (KT, 128, 2 * 128))
    inp("qbias", (128, HPC), F32)
    inp("kbias", (128, HPC), F32)
    inp("vbias_bc", (128, 2 * 128), F32)
    inp("wo_c", (H, 128, D))
    inp("wo_d", (H, 128, D))
    inp("obias_r", (128, OB), F32)
    inp("obias_i", (128, OB), F32)
    inp("w1_r", (KT, 128, HIDDEN))
    inp("w1_i", (KT, 128, HIDDEN))
    inp("bias1_r", (128, HB), F32)
    inp("bias1_i", (128, HB), F32)
    inp("modb", (128, HB), F32)
    inp("w2_r", (HB, 128, D))
    inp("w2_i", (HB, 128, D))
    inp("w2_i_neg", (HB, 128, D))
    inp("bias2_r", (128, OB), F32)
    inp("bias2_i", (128, OB), F32)
    inp("cos2", (128, L))
    inp("sin2", (128, L))
    inp("mask01", (128, 128))
    T["outT_r"] = nc.dram_tensor("outT_r", [D, TOK], F32, kind="ExternalOutput")
    T["outT_i"] = nc.dram_tensor("outT_i", [D, TOK], F32, kind="ExternalOutput")

    with tile.TileContext(nc) as tc:
        _emit(tc, T)
    nc.compile()
    return nc


# =====================================================================
# Host-side input prep
# =====================================================================
def _prep(inputs):
    f32 = np.float32
    f16 = np.float16
    g1 = (np.asarray(inputs["ln1_gr"], f32) + 1j * np.asarray(inputs["ln1_gi"], f32)).astype(np.complex128)
    b1ln = (np.asarray(inputs["ln1_br"], f32) + 1j * np.asarray(inputs["ln1_bi"], f32)).astype(np.complex128)
    g2 = (np.asarray(inputs["ln2_gr"], f32) + 1j * np.asarray(inputs["ln2_gi"], f32)).astype(np.complex128)
    b2ln = (np.asarray(inputs["ln2_br"], f32) + 1j * np.asarray(inputs["ln2_bi"], f32)).astype(np.complex128)

    def cmat(r, i):
        return (np.asarray(inputs[r], f32) + 1j * np.asarray(inputs[i], f32)).astype(np.complex128)

    Wq = cmat("Wq_r", "Wq_i")
    Wk = cmat("Wk_r", "Wk_i")
    Wv = cmat("Wv_r", "Wv_i")
    Wo = cmat("Wo_r", "Wo_i")
    W1 = cmat("W1_r", "W1_i")
    W2 = cmat("W2_r", "W2_i")
    bo = (np.asarray(inputs["bo_r"], f32) + 1j * np.asarray(inputs["bo_i"], f32)).astype(np.complex128)
    b1fc = (np.asarray(inputs["b1_r"], f32) + 1j * np.asarray(inputs["b1_i"], f32)).astype(np.complex128)
    b2fc = (np.asarray(inputs["b2_r"], f32) + 1j * np.asarray(inputs["b2_i"], f32)).astype(np.complex128)
    mod_b = np.asarray(inputs["mod_b"], f32)

    Wq_e = Wq * g1[None, :] * SCALE
    Wk_e = Wk * g1[None, :]
    Wv_e = Wv * g1[None, :]
    biasQ = (Wq @ b1ln) * SCALE
    biasK = Wk @ b1ln
    biasV = Wv @ b1ln
    W1_e = W1 * g2[None, :]
    bias1 = W1 @ b2ln + b1fc

    # RoPE tables (sign-folded sin)
    inv_freq = 1.0 / (10000.0 ** (np.arange(0, HD, 2, dtype=np.float64) / HD))  # [32]
    ang = np.arange(L, dtype=np.float64)[:, None] * inv_freq[None, :]           # [L, 32]
    cos_d = np.concatenate([np.cos(ang), np.cos(ang)], axis=1)                  # [L, 64]
    sin_d = np.concatenate([np.sin(ang), np.sin(ang)], axis=1)
    dvec = np.arange(128) % 64
    cos2 = cos_d[:, dvec].T.astype(f16)                                         # [128, L]
    sgn = np.where(dvec < 32, -1.0, 1.0)
    sin2 = (sin_d[:, dvec] * sgn[None, :]).T.astype(f16)
    mask01 = np.triu(np.ones((128, 128), dtype=f16))

    x_r = np.asarray(inputs["x_real"], f32).reshape(T_ALL, D)
    x_i = np.asarray(inputs["x_imag"], f32).reshape(T_ALL, D)

    def hsl(h):
        return slice(HD * h, HD * (h + 1))

    maps = []
    for c in range(NC):
        m = {}
        tok = slice(TOK * c, TOK * (c + 1))
        m["xT_r"] = np.ascontiguousarray(x_r[tok].T)
        m["xT_i"] = np.ascontiguousarray(x_i[tok].T)

        def qk_ab(W_e):
            a = np.empty((HPC, KT, 128, 128), f16)
            bb = np.empty((HPC, KT, 128, 128), f16)
            for hh in range(HPC):
                h = HPC * c + hh
                A = np.concatenate([W_e.real[hsl(h), :], W_e.imag[hsl(h), :]], 0).T
                Bm = np.concatenate([-W_e.imag[hsl(h), :], W_e.real[hsl(h), :]], 0).T
                a[hh] = A.reshape(KT, 128, 128)
                bb[hh] = Bm.reshape(KT, 128, 128)
            return a, bb

        m["wq_a"], m["wq_b"] = qk_ab(Wq_e)
        m["wk_a"], m["wk_b"] = qk_ab(Wk_e)
        va = np.empty((KT, 128, 2 * 128), f16)
        vb = np.empty((KT, 128, 2 * 128), f16)
        vbias = np.empty(2 * 128, f32)
        for hh in range(HPC):
            h = HPC * c + hh
            A = np.concatenate([Wv_e.real[hsl(h), :], Wv_e.imag[hsl(h), :]], 0).T
            Bm = np.concatenate([-Wv_e.imag[hsl(h), :], Wv_e.real[hsl(h), :]], 0).T
            va[:, :, 128 * hh:128 * (hh + 1)] = A.reshape(KT, 128, 128)
            vb[:, :, 128 * hh:128 * (hh + 1)] = Bm.reshape(KT, 128, 128)
            vbias[128 * hh:128 * hh + 64] = biasV.real[hsl(h)]
            vbias[128 * hh + 64:128 * (hh + 1)] = biasV.imag[hsl(h)]
        m["wv_a"], m["wv_b"] = va, vb
        m["vbias_bc"] = np.tile(vbias[None, :], (128, 1)).astype(f32)
        qb = np.empty((128, HPC), f32)
        kb = np.empty((128, HPC), f32)
        for hh in range(HPC):
            h = HPC * c + hh
            qb[:, hh] = np.concatenate([biasQ.real[hsl(h)], biasQ.imag[hsl(h)]])
            kb[:, hh] = np.concatenate([biasK.real[hsl(h)], biasK.imag[hsl(h)]])
        m["qbias"], m["kbias"] = qb, kb

        wo_c = np.empty((H, 128, D), f16)
        wo_d = np.empty((H, 128, D), f16)
        for h in range(H):
            wo_c[h] = np.concatenate([Wo.real[:, hsl(h)].T, -Wo.imag[:, hsl(h)].T], 0)
            wo_d[h] = np.concatenate([Wo.imag[:, hsl(h)].T, Wo.real[:, hsl(h)].T], 0)
        m["wo_c"], m["wo_d"] = wo_c, wo_d
        m["obias_r"] = np.ascontiguousarray(bo.real.reshape(OB, 128).T).astype(f32)
        m["obias_i"] = np.ascontiguousarray(bo.imag.reshape(OB, 128).T).astype(f32)

        m["w1_r"] = np.ascontiguousarray(W1_e.real.T).reshape(KT, 128, HIDDEN).astype(f16)
        m["w1_i"] = np.ascontiguousarray(W1_e.imag.T).reshape(KT, 128, HIDDEN).astype(f16)
        m["bias1_r"] = np.ascontiguousarray(bias1.real.reshape(HB, 128).T).astype(f32)
        m["bias1_i"] = np.ascontiguousarray(bias1.imag.reshape(HB, 128).T).astype(f32)
        m["modb"] = np.ascontiguousarray(mod_b.reshape(HB, 128).T).astype(f32)
        m["w2_r"] = np.ascontiguousarray(W2.real.T).reshape(HB, 128, D).astype(f16)
        m["w2_i"] = np.ascontiguousarray(W2.imag.T).reshape(HB, 128, D).astype(f16)
        m["w2_i_neg"] = np.ascontiguousarray(-W2.imag.T).reshape(HB, 128, D).astype(f16)
        m["bias2_r"] = np.ascontiguousarray(b2fc.real.reshape(OB, 128).T).astype(f32)
        m["bias2_i"] = np.ascontiguousarray(b2fc.imag.reshape(OB, 128).T).astype(f32)
        m["cos2"], m["sin2"], m["mask01"] = cos2, sin2, mask01
        maps.append(m)
    return maps


# =====================================================================
# Entry point
# =====================================================================
def kernel(**inputs):
    if "nc" not in _cache:
        _cache["nc"] = _build()
    nc = _cache["nc"]
    in_maps = _prep(inputs)
    res = run_bass_kernel_spmd(nc, in_maps, core_ids=list(range(NC)))
    out_r = np.empty((T_ALL, D), np.float32)
    out_i = np.empty((T_ALL, D), np.float32)
    for c in range(NC):
        out_r[TOK * c:TOK * (c + 1), :] = res.results[c]["outT_r"].T
        out_i[TOK * c:TOK * (c + 1), :] = res.results[c]["outT_i"].T
    return out_r.reshape(B, L, D), out_i.reshape(B, L, D)
